# revision 8
# baseline (speedup 1.0000x reference)
"""CRF forward kernel for Trainium2 (8 NeuronCores, Bass/Tile).

Problem: feats (256,256,52) f32, mask (256,256) bool, transitions (52,52) f32.
Reference returns (Z, scores) with
    scores[t,b,i,j] = feats[b,t,j] + transitions[i,j]        (256,256,52,52) f32
    Z = sum_b logZ_b from the CRF forward recursion.

Strategy (per core, SPMD over 8 cores):
  * scores sharded over t (32 timesteps x all 256 batch = 8192 (t,b) "pairs"/core).
    Generated on the TensorEngine as a single bf16 matmul per chunk (k=107):
        out[p, i*52+j] = f[p,j] + trans[i,j]
        lhsT=[fT_hi; fT_mid; 1;1;1] (107,128)  rhs=[I..I; I..I; t_hi;t_mid;t_lo]
    f is split 2-way bf16 (residual <= 2^-17|f| ~ 4e-5, far below the fp32 ulp
    of the +-10000 trans scale), trans 3-way bf16 (exact); identity/ones
    weights exact; PSUM accumulates fp32 -> within ~1-2 ulp of fp32(f+t).
    Single matmuls (no start/stop accumulation pairs) keep the PE at the warm
    2.4 GHz clock -- accumulation pairs measured 2.5x slower issue rate.
    PSUM tiles go through a ScalarE/DVE copy to SBUF, then DMA to HBM.
  * CRF scan sharded over b (32 batch/core), run in exp space:
        q_t = expf_t * (A^T @ q_{t-1}),  A = exp(trans)
    where the host pre-computes per-(b,t) log-offsets so q stays in fp32 range,
    expf_t[j,b] = exp(f[b,t,j] - delta[b,t]).  One 52x52x32 matmul + one DVE
    multiply per step; all 255 q_t are kept in an SBUF history buffer and DMA'd
    out once.  The mask never touches the device: lengths are a prefix, so the
    host just reads q at t = len[b]-1 and finishes Z in float64.
"""

import sys
import types

for _p in ("/opt/trn_rl_repo",):
    if _p not in sys.path:
        sys.path.append(_p)

import numpy as np
import ml_dtypes

# ---------------------------------------------------------------- NTFF hook
# The image's antenv lacks axon_hooks; recreate it so trace=True can profile.
def _install_profile_hook():
    if "antenv.axon_hooks" in sys.modules:
        return
    mod = types.ModuleType("antenv.axon_hooks")
    holder = [None]
    mod.set_axon_ntff_profile_hook = lambda h: holder.__setitem__(0, h)
    mod.get_axon_ntff_profile_hook = lambda: holder[0]
    sys.modules["antenv.axon_hooks"] = mod
    try:
        import antenv

        antenv.axon_hooks = mod
        from trn_agent_boot.trn_boot import _ntff_profile_via_ctypes

        mod.set_axon_ntff_profile_hook(
            _ntff_profile_via_ctypes("/opt/axon/libaxon_pjrt.so")
        )
    except Exception:
        pass


_install_profile_hook()

import concourse.bacc as bacc
import concourse.mybir as mybir
import concourse.tile as tile
from concourse.bass_utils import run_bass_kernel_spmd

BF16 = ml_dtypes.bfloat16
B, S, T = 256, 256, 52
NCORES = 8
TSH = S // NCORES          # 32 timesteps per core (scores shard)
BSH = B // NCORES          # 32 batch per core (scan shard)
PAIRS = TSH * B            # 8192 scores pairs per core
GROUP = 128                # pairs per matmul group
NG = PAIRS // GROUP        # 64
TILE = T * T               # 2704
CHUNKS = [(c, min(512, TILE - c)) for c in range(0, TILE, 512)]
START_TAG, STOP_TAG = T - 2, T - 1

# test.py hooks
TRACE = {"trace": False}
LAST_RESULT = [None]

_BUILT = [None]


def _build_nc():
    if _BUILT[0] is not None:
        return _BUILT[0]
    nc = bacc.Bacc("TRN2", target_bir_lowering=False, debug=False)
    f32, bf16, f16 = mybir.dt.float32, mybir.dt.bfloat16, mybir.dt.float16

    # row sizes padded to DMA-friendly (power-of-two bytes) shapes
    d_fts = nc.dram_tensor("fts", [107, PAIRS], bf16, kind="ExternalInput")
    d_rhsc = nc.dram_tensor("rhsc", [107, 4096], bf16, kind="ExternalInput")
    d_scanmisc = nc.dram_tensor("scanmisc", [T, 1024], f16, kind="ExternalInput")
    d_expf = nc.dram_tensor("expf", [T, 8192], f16, kind="ExternalInput")
    d_scores = nc.dram_tensor("scores", [PAIRS, TILE], f32, kind="ExternalOutput")
    d_hist = nc.dram_tensor("hist", [T, S * BSH], f16, kind="ExternalOutput")

    with tile.TileContext(nc) as tc:
        with (
            tc.tile_pool(name="const", bufs=1) as const,
            tc.tile_pool(name="ps", bufs=6, space="PSUM") as pspool,
            tc.tile_pool(name="scanps", bufs=2, space="PSUM") as scanps,
            tc.tile_pool(name="stage", bufs=8) as stage,
        ):
            fts_sb = const.tile([107, PAIRS], bf16)
            nc.sync.dma_start(fts_sb[:], d_fts[:])
            rhsc_sb = const.tile([107, 4096], bf16)
            nc.gpsimd.dma_start(rhsc_sb[:], d_rhsc[:])
            scanmisc_sb = const.tile([T, 1024], f16)
            nc.gpsimd.dma_start(scanmisc_sb[:], d_scanmisc[:])
            expf_sb = const.tile([T, 8192], f16)
            nc.scalar.dma_start(expf_sb[:], d_expf[:])
            aexp_sb = scanmisc_sb[:, 0:T]
            q0_sb = scanmisc_sb[:, 512 : 512 + BSH]
            hist_sb = const.tile([T, S * BSH], f16)

            scan_t = 1

            def scan_step():
                nonlocal scan_t
                if scan_t >= S:
                    return
                t = scan_t
                scan_t += 1
                sp = scanps.tile([T, BSH], f32)
                prev = q0_sb if t == 1 else hist_sb[:, (t - 1) * BSH : t * BSH]
                nc.tensor.matmul(sp[:], aexp_sb, prev, start=True, stop=True)
                nc.vector.tensor_mul(
                    hist_sb[:, t * BSH : (t + 1) * BSH], sp[:],
                    expf_sb[:, (t - 1) * BSH : t * BSH],
                )

            for g in range(NG):
                gsl = slice(g * GROUP, (g + 1) * GROUP)
                for ci, (c0, cn) in enumerate(CHUNKS):
                    ps = pspool.tile([GROUP, 512], f32)
                    nc.tensor.matmul(
                        ps[:, :cn], fts_sb[:, gsl], rhsc_sb[:, c0 : c0 + cn],
                        start=True, stop=True,
                    )
                    # space scan steps so their DVE dependency clears before
                    # the scan matmul reaches the PE FIFO head
                    if ci in (1, 2, 4, 5):
                        scan_step()
                    st = stage.tile([GROUP, 512], f32)
                    if ci % 2 == 1:
                        nc.vector.tensor_copy(st[:, :cn], ps[:, :cn])
                    else:
                        nc.scalar.copy(st[:, :cn], ps[:, :cn])
                    dmae = (nc.sync, nc.gpsimd, nc.scalar)[ci % 3]
                    dmae.dma_start(d_scores[gsl, c0 : c0 + cn], st[:, :cn])
            nc.sync.dma_start(d_hist[:], hist_sb[:])
    nc.finalize()
    _BUILT[0] = nc
    return nc


def _split3(x32):
    """Exact 3-way bf16 split of float32: hi+mid+lo == x exactly."""
    hi = x32.astype(BF16)
    r1 = x32 - hi.astype(np.float32)
    mid = r1.astype(BF16)
    r2 = r1 - mid.astype(np.float32)
    lo = r2.astype(BF16)
    return hi, mid, lo


def _logsumexp64(x, axis=-1):
    m = np.max(x, axis=axis, keepdims=True)
    m = np.where(np.isfinite(m), m, 0.0)
    with np.errstate(divide="ignore"):
        return np.squeeze(m, axis) + np.log(
            np.sum(np.exp(x - m), axis=axis)
        )


def kernel(feats, mask, transitions):
    feats = np.ascontiguousarray(np.asarray(feats), dtype=np.float32)
    mask = np.asarray(mask)
    trans = np.ascontiguousarray(np.asarray(transitions), dtype=np.float32)
    assert feats.shape == (B, S, T) and trans.shape == (T, T)

    nc = _build_nc()

    # ---- scores-side host prep (t-shard) -------------------------------
    f_hi, f_mid, _ = _split3(feats)             # (B,S,T) bf16 (2 parts used)
    idx = np.arange(TILE)
    iblk = np.zeros((T, TILE), dtype=BF16)
    iblk[idx % T, idx] = 1.0                     # [I I ... I] (52,2704)
    t_hi, t_mid, t_lo = _split3(trans.reshape(-1))
    rhsc = np.zeros((107, 4096), dtype=BF16)
    rhsc[:, :TILE] = np.concatenate(
        [iblk, iblk, t_hi[None], t_mid[None], t_lo[None]], axis=0
    )

    # ---- scan-side host prep (b-shard) ---------------------------------
    f64 = np.float64
    with np.errstate(over="ignore"):
        A = np.exp(trans.astype(f64))
    A16 = A.astype(np.float16)
    with np.errstate(divide="ignore"):
        logc = np.log(A.sum(axis=0))             # (T,) -inf on dead columns
    # per-(b,t) growth offset: delta[b,t] ~ log-growth of sum_j q; the
    # softmax-weighted column average is ~colsum/n_live, hence -log(50).
    delta = _logsumexp64(
        feats.astype(f64) + logc[None, None, :], axis=2
    ) - np.log(50.0)                                                        # (B,S)
    p0 = feats[:, 0, :].astype(f64) + trans[START_TAG].astype(f64)          # (B,T)
    o0 = _logsumexp64(p0, axis=1)                                           # (B,)
    ocum = np.concatenate(
        [o0[:, None], o0[:, None] + np.cumsum(delta[:, 1:], axis=1)], axis=1
    )                                                                       # (B,S)
    with np.errstate(under="ignore"):
        expf = np.exp(
            feats[:, 1:, :].astype(f64) - delta[:, 1:, None]
        ).astype(np.float16)                                                # (B,S-1,T)
        q0 = np.exp(p0 - o0[:, None]).astype(np.float16)                    # (B,T)

    in_maps = []
    for k in range(NCORES):
        tsl = slice(TSH * k, TSH * (k + 1))
        bsl = slice(BSH * k, BSH * (k + 1))

        def arr(a):  # (B,TSH,T) -> (T, TSH*B) with pair index t_local*B + b
            return np.ascontiguousarray(
                a[:, tsl, :].transpose(2, 1, 0).reshape(T, PAIRS)
            )

        fts = np.concatenate(
            [arr(f_hi), arr(f_mid), np.ones((3, PAIRS), dtype=BF16)], axis=0
        )
        expf_k = np.zeros((T, 8192), dtype=np.float16)
        expf_k[:, : (S - 1) * BSH] = (
            expf[bsl].transpose(2, 1, 0).reshape(T, (S - 1) * BSH)
        )
        scanmisc_k = np.zeros((T, 1024), dtype=np.float16)
        scanmisc_k[:, :T] = A16
        scanmisc_k[:, 512 : 512 + BSH] = q0[bsl].T
        in_maps.append(
            {
                "fts": fts,
                "rhsc": rhsc,
                "scanmisc": scanmisc_k,
                "expf": expf_k,
            }
        )

    res = run_bass_kernel_spmd(
        nc, in_maps, list(range(NCORES)), trace=TRACE["trace"]
    )
    LAST_RESULT[0] = res

    # ---- assemble scores ----------------------------------------------
    scores = np.empty((S, B, T, T), dtype=np.float32)
    for k in range(NCORES):
        scores[TSH * k : TSH * (k + 1)] = res.results[k]["scores"].reshape(
            TSH, B, T, T
        )

    # ---- finish Z on host ---------------------------------------------
    lengths = mask.reshape(B, S).sum(axis=1).astype(np.int64)
    tcol = trans[:, STOP_TAG].astype(f64)
    Z = 0.0
    for k in range(NCORES):
        hist = res.results[k]["hist"]            # (T, S*BSH)
        for bl in range(BSH):
            b = BSH * k + bl
            tstar = int(lengths[b]) - 1
            if tstar == 0:
                pS = p0[b]
            else:
                q = hist[:, tstar * BSH + bl].astype(f64)
                with np.errstate(divide="ignore"):
                    pS = np.log(q) + ocum[b, tstar]
            Z += _logsumexp64(pS + tcol, axis=0)
    return np.float32(Z), scores


# revision 9
# speedup vs baseline: 1.0051x; 1.0051x over previous
"""CRF forward kernel for Trainium2 (8 NeuronCores, Bass/Tile).

Problem: feats (256,256,52) f32, mask (256,256) bool, transitions (52,52) f32.
Reference returns (Z, scores) with
    scores[t,b,i,j] = feats[b,t,j] + transitions[i,j]        (256,256,52,52) f32
    Z = sum_b logZ_b from the CRF forward recursion.

Strategy (per core, SPMD over 8 cores):
  * scores sharded over t (32 timesteps x all 256 batch = 8192 (t,b) "pairs"/core).
    Generated on the TensorEngine as a single bf16 matmul per chunk (k=107):
        out[p, i*52+j] = f[p,j] + trans[i,j]
        lhsT=[fT_hi; fT_mid; 1;1;1] (107,128)  rhs=[I..I; I..I; t_hi;t_mid;t_lo]
    f is split 2-way bf16 (residual <= 2^-17|f| ~ 4e-5, far below the fp32 ulp
    of the +-10000 trans scale), trans 3-way bf16 (exact); identity/ones
    weights exact; PSUM accumulates fp32 -> within ~1-2 ulp of fp32(f+t).
    Single matmuls (no start/stop accumulation pairs) keep the PE at the warm
    2.4 GHz clock -- accumulation pairs measured 2.5x slower issue rate.
    PSUM tiles go through a ScalarE/DVE copy to SBUF, then DMA to HBM.
  * CRF scan sharded over b (32 batch/core), run in exp space:
        q_t = expf_t * (A^T @ q_{t-1}),  A = exp(trans)
    where the host pre-computes per-(b,t) log-offsets so q stays in fp32 range,
    expf_t[j,b] = exp(f[b,t,j] - delta[b,t]).  One 52x52x32 matmul + one DVE
    multiply per step; all 255 q_t are kept in an SBUF history buffer and DMA'd
    out once.  The mask never touches the device: lengths are a prefix, so the
    host just reads q at t = len[b]-1 and finishes Z in float64.
"""

import sys
import types

for _p in ("/opt/trn_rl_repo",):
    if _p not in sys.path:
        sys.path.append(_p)

import numpy as np
import ml_dtypes

# ---------------------------------------------------------------- NTFF hook
# The image's antenv lacks axon_hooks; recreate it so trace=True can profile.
def _install_profile_hook():
    if "antenv.axon_hooks" in sys.modules:
        return
    mod = types.ModuleType("antenv.axon_hooks")
    holder = [None]
    mod.set_axon_ntff_profile_hook = lambda h: holder.__setitem__(0, h)
    mod.get_axon_ntff_profile_hook = lambda: holder[0]
    sys.modules["antenv.axon_hooks"] = mod
    try:
        import antenv

        antenv.axon_hooks = mod
        from trn_agent_boot.trn_boot import _ntff_profile_via_ctypes

        mod.set_axon_ntff_profile_hook(
            _ntff_profile_via_ctypes("/opt/axon/libaxon_pjrt.so")
        )
    except Exception:
        pass


_install_profile_hook()

import concourse.bacc as bacc
import concourse.mybir as mybir
import concourse.tile as tile
from concourse.bass_utils import run_bass_kernel_spmd

BF16 = ml_dtypes.bfloat16
B, S, T = 256, 256, 52
NCORES = 8
TSH = S // NCORES          # 32 timesteps per core (scores shard)
BSH = B // NCORES          # 32 batch per core (scan shard)
PAIRS = TSH * B            # 8192 scores pairs per core
GROUP = 128                # pairs per matmul group
NG = PAIRS // GROUP        # 64
TILE = T * T               # 2704
CHUNKS = [(c, min(512, TILE - c)) for c in range(0, TILE, 512)]
START_TAG, STOP_TAG = T - 2, T - 1

# test.py hooks
TRACE = {"trace": False}
LAST_RESULT = [None]

_BUILT = [None]


def _build_nc():
    if _BUILT[0] is not None:
        return _BUILT[0]
    nc = bacc.Bacc("TRN2", target_bir_lowering=False, debug=False)
    f32, bf16, f16 = mybir.dt.float32, mybir.dt.bfloat16, mybir.dt.float16

    # row sizes padded to DMA-friendly (power-of-two bytes) shapes
    d_fts = nc.dram_tensor("fts", [107, PAIRS], bf16, kind="ExternalInput")
    d_rhsc = nc.dram_tensor("rhsc", [107, 4096], bf16, kind="ExternalInput")
    d_scanmisc = nc.dram_tensor("scanmisc", [T, 1024], f16, kind="ExternalInput")
    d_expf = nc.dram_tensor("expf", [T, 8192], f16, kind="ExternalInput")
    d_scores = nc.dram_tensor("scores", [PAIRS, TILE], f32, kind="ExternalOutput")
    d_hist = nc.dram_tensor("hist", [T, S * BSH], f16, kind="ExternalOutput")

    with tile.TileContext(nc) as tc:
        with (
            tc.tile_pool(name="const", bufs=1) as const,
            tc.tile_pool(name="ps", bufs=6, space="PSUM") as pspool,
            tc.tile_pool(name="scanps", bufs=2, space="PSUM") as scanps,
            tc.tile_pool(name="stage", bufs=8) as stage,
        ):
            fts_sb = const.tile([107, PAIRS], bf16)
            nc.sync.dma_start(fts_sb[:], d_fts[:])
            rhsc_sb = const.tile([107, 4096], bf16)
            nc.scalar.dma_start(rhsc_sb[:], d_rhsc[:])
            scanmisc_sb = const.tile([T, 1024], f16)
            nc.sync.dma_start(scanmisc_sb[:], d_scanmisc[:])
            expf_sb = const.tile([T, 8192], f16)
            nc.scalar.dma_start(expf_sb[:], d_expf[:])
            # (loads stay on Sync/Scalar HWDGE queues; the GpSimd DMA queue is
            # software-DGE and moves DRAM->SBUF loads at <1 GB/s)
            aexp_sb = scanmisc_sb[:, 0:T]
            q0_sb = scanmisc_sb[:, 512 : 512 + BSH]
            hist_sb = const.tile([T, S * BSH], f16)

            scan_t = 1

            def scan_step():
                nonlocal scan_t
                if scan_t >= S:
                    return
                t = scan_t
                scan_t += 1
                sp = scanps.tile([T, BSH], f32)
                prev = q0_sb if t == 1 else hist_sb[:, (t - 1) * BSH : t * BSH]
                nc.tensor.matmul(sp[:], aexp_sb, prev, start=True, stop=True)
                nc.vector.tensor_mul(
                    hist_sb[:, t * BSH : (t + 1) * BSH], sp[:],
                    expf_sb[:, (t - 1) * BSH : t * BSH],
                )

            for g in range(NG):
                gsl = slice(g * GROUP, (g + 1) * GROUP)
                for ci, (c0, cn) in enumerate(CHUNKS):
                    ps = pspool.tile([GROUP, 512], f32)
                    nc.tensor.matmul(
                        ps[:, :cn], fts_sb[:, gsl], rhsc_sb[:, c0 : c0 + cn],
                        start=True, stop=True,
                    )
                    # space scan steps so their DVE dependency clears before
                    # the scan matmul reaches the PE FIFO head
                    if ci in (1, 2, 4, 5):
                        scan_step()
                    st = stage.tile([GROUP, 512], f32)
                    if ci % 2 == 1:
                        nc.vector.tensor_copy(st[:, :cn], ps[:, :cn])
                    else:
                        nc.scalar.copy(st[:, :cn], ps[:, :cn])
                    dmae = (nc.sync, nc.gpsimd, nc.scalar)[ci % 3]
                    dmae.dma_start(d_scores[gsl, c0 : c0 + cn], st[:, :cn])
            nc.sync.dma_start(d_hist[:], hist_sb[:])
    nc.finalize()
    _BUILT[0] = nc
    return nc


def _split3(x32):
    """Exact 3-way bf16 split of float32: hi+mid+lo == x exactly."""
    hi = x32.astype(BF16)
    r1 = x32 - hi.astype(np.float32)
    mid = r1.astype(BF16)
    r2 = r1 - mid.astype(np.float32)
    lo = r2.astype(BF16)
    return hi, mid, lo


def _logsumexp64(x, axis=-1):
    m = np.max(x, axis=axis, keepdims=True)
    m = np.where(np.isfinite(m), m, 0.0)
    with np.errstate(divide="ignore"):
        return np.squeeze(m, axis) + np.log(
            np.sum(np.exp(x - m), axis=axis)
        )


def kernel(feats, mask, transitions):
    feats = np.ascontiguousarray(np.asarray(feats), dtype=np.float32)
    mask = np.asarray(mask)
    trans = np.ascontiguousarray(np.asarray(transitions), dtype=np.float32)
    assert feats.shape == (B, S, T) and trans.shape == (T, T)

    nc = _build_nc()

    # ---- scores-side host prep (t-shard) -------------------------------
    f_hi, f_mid, _ = _split3(feats)             # (B,S,T) bf16 (2 parts used)
    idx = np.arange(TILE)
    iblk = np.zeros((T, TILE), dtype=BF16)
    iblk[idx % T, idx] = 1.0                     # [I I ... I] (52,2704)
    t_hi, t_mid, t_lo = _split3(trans.reshape(-1))
    rhsc = np.zeros((107, 4096), dtype=BF16)
    rhsc[:, :TILE] = np.concatenate(
        [iblk, iblk, t_hi[None], t_mid[None], t_lo[None]], axis=0
    )

    # ---- scan-side host prep (b-shard) ---------------------------------
    f64 = np.float64
    with np.errstate(over="ignore"):
        A = np.exp(trans.astype(f64))
    A16 = A.astype(np.float16)
    with np.errstate(divide="ignore"):
        logc = np.log(A.sum(axis=0))             # (T,) -inf on dead columns
    # per-(b,t) growth offset: delta[b,t] ~ log-growth of sum_j q; the
    # softmax-weighted column average is ~colsum/n_live, hence -log(50).
    delta = _logsumexp64(
        feats.astype(f64) + logc[None, None, :], axis=2
    ) - np.log(50.0)                                                        # (B,S)
    p0 = feats[:, 0, :].astype(f64) + trans[START_TAG].astype(f64)          # (B,T)
    o0 = _logsumexp64(p0, axis=1)                                           # (B,)
    ocum = np.concatenate(
        [o0[:, None], o0[:, None] + np.cumsum(delta[:, 1:], axis=1)], axis=1
    )                                                                       # (B,S)
    with np.errstate(under="ignore"):
        expf = np.exp(
            feats[:, 1:, :].astype(f64) - delta[:, 1:, None]
        ).astype(np.float16)                                                # (B,S-1,T)
        q0 = np.exp(p0 - o0[:, None]).astype(np.float16)                    # (B,T)

    in_maps = []
    for k in range(NCORES):
        tsl = slice(TSH * k, TSH * (k + 1))
        bsl = slice(BSH * k, BSH * (k + 1))

        def arr(a):  # (B,TSH,T) -> (T, TSH*B) with pair index t_local*B + b
            return np.ascontiguousarray(
                a[:, tsl, :].transpose(2, 1, 0).reshape(T, PAIRS)
            )

        fts = np.concatenate(
            [arr(f_hi), arr(f_mid), np.ones((3, PAIRS), dtype=BF16)], axis=0
        )
        expf_k = np.zeros((T, 8192), dtype=np.float16)
        expf_k[:, : (S - 1) * BSH] = (
            expf[bsl].transpose(2, 1, 0).reshape(T, (S - 1) * BSH)
        )
        scanmisc_k = np.zeros((T, 1024), dtype=np.float16)
        scanmisc_k[:, :T] = A16
        scanmisc_k[:, 512 : 512 + BSH] = q0[bsl].T
        in_maps.append(
            {
                "fts": fts,
                "rhsc": rhsc,
                "scanmisc": scanmisc_k,
                "expf": expf_k,
            }
        )

    res = run_bass_kernel_spmd(
        nc, in_maps, list(range(NCORES)), trace=TRACE["trace"]
    )
    LAST_RESULT[0] = res

    # ---- assemble scores ----------------------------------------------
    scores = np.empty((S, B, T, T), dtype=np.float32)
    for k in range(NCORES):
        scores[TSH * k : TSH * (k + 1)] = res.results[k]["scores"].reshape(
            TSH, B, T, T
        )

    # ---- finish Z on host ---------------------------------------------
    lengths = mask.reshape(B, S).sum(axis=1).astype(np.int64)
    tcol = trans[:, STOP_TAG].astype(f64)
    Z = 0.0
    for k in range(NCORES):
        hist = res.results[k]["hist"]            # (T, S*BSH)
        for bl in range(BSH):
            b = BSH * k + bl
            tstar = int(lengths[b]) - 1
            if tstar == 0:
                pS = p0[b]
            else:
                q = hist[:, tstar * BSH + bl].astype(f64)
                with np.errstate(divide="ignore"):
                    pS = np.log(q) + ocum[b, tstar]
            Z += _logsumexp64(pS + tcol, axis=0)
    return np.float32(Z), scores


# revision 11
# speedup vs baseline: 1.0548x; 1.0495x over previous
"""CRF forward kernel for Trainium2 (8 NeuronCores, Bass/Tile).

Problem: feats (256,256,52) f32, mask (256,256) bool, transitions (52,52) f32.
Reference returns (Z, scores) with
    scores[t,b,i,j] = feats[b,t,j] + transitions[i,j]        (256,256,52,52) f32
    Z = sum_b logZ_b from the CRF forward recursion.

Strategy (per core, SPMD over 8 cores):
  * scores sharded over t (32 timesteps x all 256 batch = 8192 (t,b) "pairs"/core).
    Generated on the TensorEngine as a single bf16 matmul per chunk (k=107):
        out[p, i*52+j] = f[p,j] + trans[i,j]
        lhsT=[fT_hi; fT_mid; 1;1;1] (107,128)  rhs=[I..I; I..I; t_hi;t_mid;t_lo]
    f is split 2-way bf16 (residual <= 2^-17|f| ~ 4e-5, far below the fp32 ulp
    of the +-10000 trans scale), trans 3-way bf16 (exact); identity/ones
    weights exact; PSUM accumulates fp32 -> within ~1-2 ulp of fp32(f+t).
    Single matmuls (no start/stop accumulation pairs) keep the PE at the warm
    2.4 GHz clock -- accumulation pairs measured 2.5x slower issue rate.
    PSUM tiles go through a ScalarE/DVE copy to SBUF, then DMA to HBM.
  * CRF scan sharded over b (32 batch/core), run in exp space:
        q_t = expf_t * (A^T @ q_{t-1}),  A = exp(trans)
    where the host pre-computes per-(b,t) log-offsets so q stays in fp32 range,
    expf_t[j,b] = exp(f[b,t,j] - delta[b,t]).  One 52x52x32 matmul + one DVE
    multiply per step; all 255 q_t are kept in an SBUF history buffer and DMA'd
    out once.  The mask never touches the device: lengths are a prefix, so the
    host just reads q at t = len[b]-1 and finishes Z in float64.
"""

import sys
import types

for _p in ("/opt/trn_rl_repo",):
    if _p not in sys.path:
        sys.path.append(_p)

import numpy as np
import ml_dtypes

# ---------------------------------------------------------------- NTFF hook
# The image's antenv lacks axon_hooks; recreate it so trace=True can profile.
def _install_profile_hook():
    if "antenv.axon_hooks" in sys.modules:
        return
    mod = types.ModuleType("antenv.axon_hooks")
    holder = [None]
    mod.set_axon_ntff_profile_hook = lambda h: holder.__setitem__(0, h)
    mod.get_axon_ntff_profile_hook = lambda: holder[0]
    sys.modules["antenv.axon_hooks"] = mod
    try:
        import antenv

        antenv.axon_hooks = mod
        from trn_agent_boot.trn_boot import _ntff_profile_via_ctypes

        mod.set_axon_ntff_profile_hook(
            _ntff_profile_via_ctypes("/opt/axon/libaxon_pjrt.so")
        )
    except Exception:
        pass


_install_profile_hook()

import concourse.bacc as bacc
import concourse.mybir as mybir
import concourse.tile as tile
from concourse.bass_utils import run_bass_kernel_spmd

BF16 = ml_dtypes.bfloat16
B, S, T = 256, 256, 52
NCORES = 8
TSH = S // NCORES          # 32 timesteps per core (scores shard)
BSH = B // NCORES          # 32 batch per core (scan shard)
PAIRS = TSH * B            # 8192 scores pairs per core
GROUP = 128                # pairs per matmul group
NG = PAIRS // GROUP        # 64
TILE = T * T               # 2704
CHUNKS = [(c, min(512, TILE - c)) for c in range(0, TILE, 512)]
START_TAG, STOP_TAG = T - 2, T - 1

# test.py hooks
TRACE = {"trace": False}
LAST_RESULT = [None]

_BUILT = [None]


def _build_nc():
    if _BUILT[0] is not None:
        return _BUILT[0]
    nc = bacc.Bacc("TRN2", target_bir_lowering=False, debug=False)
    f32, bf16, f16 = mybir.dt.float32, mybir.dt.bfloat16, mybir.dt.float16

    # row sizes padded to DMA-friendly (power-of-two bytes) shapes
    d_fts = nc.dram_tensor("fts", [107, PAIRS], bf16, kind="ExternalInput")
    d_rhsc = nc.dram_tensor("rhsc", [107, 4096], bf16, kind="ExternalInput")
    d_scanmisc = nc.dram_tensor("scanmisc", [T, 1024], f16, kind="ExternalInput")
    d_expf = nc.dram_tensor("expf", [T, 8192], f16, kind="ExternalInput")
    d_scores = nc.dram_tensor("scores", [PAIRS, TILE], f32, kind="ExternalOutput")
    d_hist = nc.dram_tensor("hist", [T, S * BSH], f16, kind="ExternalOutput")

    with tile.TileContext(nc) as tc:
        with (
            tc.tile_pool(name="const", bufs=1) as const,
            tc.tile_pool(name="ps", bufs=6, space="PSUM") as pspool,
            tc.tile_pool(name="scanps", bufs=2, space="PSUM") as scanps,
            tc.tile_pool(name="stage", bufs=12) as stage,
        ):
            # HBM->SBUF loads run at ~25-40 GB/s here (stores are ~10x
            # faster), so loads are sliced into separate tiles and streamed
            # just-in-time behind compute instead of front-loaded.
            scanmisc_sb = const.tile([T, 1024], f16)
            nc.sync.dma_start(scanmisc_sb[:], d_scanmisc[:])
            aexp_sb = scanmisc_sb[:, 0:T]
            q0_sb = scanmisc_sb[:, 512 : 512 + BSH]

            rhsc_t = []
            for ci, (c0, cn) in enumerate(CHUNKS):
                rt = const.tile([107, 512], bf16, tag=f"rhsc{ci}")
                nc.scalar.dma_start(rt[:, :cn], d_rhsc[:, c0 : c0 + cn])
                rhsc_t.append(rt)

            expf_sb = const.tile([T, 8192], f16)
            for i in range(8):
                sl = slice(i * 1024, (i + 1) * 1024)
                nc.scalar.dma_start(expf_sb[:, sl], d_expf[:, sl])

            hist_sb = const.tile([T, S * BSH], f16)

            # fts in 8 separate tiles of 8 groups each, prefetched ahead
            FGRP = 8
            fts_t = [None] * FGRP
            def load_fts(k):
                ft = const.tile([107, GROUP * FGRP], bf16, tag=f"fts{k}")
                sl = slice(k * GROUP * FGRP, (k + 1) * GROUP * FGRP)
                nc.sync.dma_start(ft[:], d_fts[:, sl])
                fts_t[k] = ft

            load_fts(0)
            load_fts(1)

            scan_t = 1

            def scan_step():
                nonlocal scan_t
                if scan_t >= S:
                    return
                t = scan_t
                scan_t += 1
                sp = scanps.tile([T, BSH], f32)
                prev = q0_sb if t == 1 else hist_sb[:, (t - 1) * BSH : t * BSH]
                nc.tensor.matmul(sp[:], aexp_sb, prev, start=True, stop=True)
                nc.vector.tensor_mul(
                    hist_sb[:, t * BSH : (t + 1) * BSH], sp[:],
                    expf_sb[:, (t - 1) * BSH : t * BSH],
                )

            for g in range(NG):
                if g % FGRP == 0 and g // FGRP + 2 < FGRP:
                    load_fts(g // FGRP + 2)
                lhs = fts_t[g // FGRP][:, (g % FGRP) * GROUP : (g % FGRP + 1) * GROUP]
                for ci, (c0, cn) in enumerate(CHUNKS):
                    ps = pspool.tile([GROUP, 512], f32)
                    nc.tensor.matmul(
                        ps[:, :cn], lhs, rhsc_t[ci][:, :cn],
                        start=True, stop=True,
                    )
                    # scan starts after expf has streamed in (~group 16)
                    if g >= 16:
                        scan_step()
                    st = stage.tile([GROUP, 512], f32)
                    if ci % 2 == 1:
                        nc.vector.tensor_copy(st[:, :cn], ps[:, :cn])
                    else:
                        nc.scalar.copy(st[:, :cn], ps[:, :cn])
                    dmae = (nc.sync, nc.gpsimd, nc.scalar)[ci % 3]
                    dmae.dma_start(
                        d_scores[g * GROUP : (g + 1) * GROUP, c0 : c0 + cn],
                        st[:, :cn],
                    )
            nc.sync.dma_start(d_hist[:], hist_sb[:])
    nc.finalize()
    _BUILT[0] = nc
    return nc


def _split3(x32):
    """Exact 3-way bf16 split of float32: hi+mid+lo == x exactly."""
    hi = x32.astype(BF16)
    r1 = x32 - hi.astype(np.float32)
    mid = r1.astype(BF16)
    r2 = r1 - mid.astype(np.float32)
    lo = r2.astype(BF16)
    return hi, mid, lo


def _logsumexp64(x, axis=-1):
    m = np.max(x, axis=axis, keepdims=True)
    m = np.where(np.isfinite(m), m, 0.0)
    with np.errstate(divide="ignore"):
        return np.squeeze(m, axis) + np.log(
            np.sum(np.exp(x - m), axis=axis)
        )


def kernel(feats, mask, transitions):
    feats = np.ascontiguousarray(np.asarray(feats), dtype=np.float32)
    mask = np.asarray(mask)
    trans = np.ascontiguousarray(np.asarray(transitions), dtype=np.float32)
    assert feats.shape == (B, S, T) and trans.shape == (T, T)

    nc = _build_nc()

    # ---- scores-side host prep (t-shard) -------------------------------
    f_hi, f_mid, _ = _split3(feats)             # (B,S,T) bf16 (2 parts used)
    idx = np.arange(TILE)
    iblk = np.zeros((T, TILE), dtype=BF16)
    iblk[idx % T, idx] = 1.0                     # [I I ... I] (52,2704)
    t_hi, t_mid, t_lo = _split3(trans.reshape(-1))
    rhsc = np.zeros((107, 4096), dtype=BF16)
    rhsc[:, :TILE] = np.concatenate(
        [iblk, iblk, t_hi[None], t_mid[None], t_lo[None]], axis=0
    )

    # ---- scan-side host prep (b-shard) ---------------------------------
    f64 = np.float64
    with np.errstate(over="ignore"):
        A = np.exp(trans.astype(f64))
    A16 = A.astype(np.float16)
    with np.errstate(divide="ignore"):
        logc = np.log(A.sum(axis=0))             # (T,) -inf on dead columns
    # per-(b,t) growth offset: delta[b,t] ~ log-growth of sum_j q; the
    # softmax-weighted column average is ~colsum/n_live, hence -log(50).
    delta = _logsumexp64(
        feats.astype(f64) + logc[None, None, :], axis=2
    ) - np.log(50.0)                                                        # (B,S)
    p0 = feats[:, 0, :].astype(f64) + trans[START_TAG].astype(f64)          # (B,T)
    o0 = _logsumexp64(p0, axis=1)                                           # (B,)
    ocum = np.concatenate(
        [o0[:, None], o0[:, None] + np.cumsum(delta[:, 1:], axis=1)], axis=1
    )                                                                       # (B,S)
    with np.errstate(under="ignore"):
        expf = np.exp(
            feats[:, 1:, :].astype(f64) - delta[:, 1:, None]
        ).astype(np.float16)                                                # (B,S-1,T)
        q0 = np.exp(p0 - o0[:, None]).astype(np.float16)                    # (B,T)

    in_maps = []
    for k in range(NCORES):
        tsl = slice(TSH * k, TSH * (k + 1))
        bsl = slice(BSH * k, BSH * (k + 1))

        def arr(a):  # (B,TSH,T) -> (T, TSH*B) with pair index t_local*B + b
            return np.ascontiguousarray(
                a[:, tsl, :].transpose(2, 1, 0).reshape(T, PAIRS)
            )

        fts = np.concatenate(
            [arr(f_hi), arr(f_mid), np.ones((3, PAIRS), dtype=BF16)], axis=0
        )
        expf_k = np.zeros((T, 8192), dtype=np.float16)
        expf_k[:, : (S - 1) * BSH] = (
            expf[bsl].transpose(2, 1, 0).reshape(T, (S - 1) * BSH)
        )
        scanmisc_k = np.zeros((T, 1024), dtype=np.float16)
        scanmisc_k[:, :T] = A16
        scanmisc_k[:, 512 : 512 + BSH] = q0[bsl].T
        in_maps.append(
            {
                "fts": fts,
                "rhsc": rhsc,
                "scanmisc": scanmisc_k,
                "expf": expf_k,
            }
        )

    res = run_bass_kernel_spmd(
        nc, in_maps, list(range(NCORES)), trace=TRACE["trace"]
    )
    LAST_RESULT[0] = res

    # ---- assemble scores ----------------------------------------------
    scores = np.empty((S, B, T, T), dtype=np.float32)
    for k in range(NCORES):
        scores[TSH * k : TSH * (k + 1)] = res.results[k]["scores"].reshape(
            TSH, B, T, T
        )

    # ---- finish Z on host ---------------------------------------------
    lengths = mask.reshape(B, S).sum(axis=1).astype(np.int64)
    tcol = trans[:, STOP_TAG].astype(f64)
    Z = 0.0
    for k in range(NCORES):
        hist = res.results[k]["hist"]            # (T, S*BSH)
        for bl in range(BSH):
            b = BSH * k + bl
            tstar = int(lengths[b]) - 1
            if tstar == 0:
                pS = p0[b]
            else:
                q = hist[:, tstar * BSH + bl].astype(f64)
                with np.errstate(divide="ignore"):
                    pS = np.log(q) + ocum[b, tstar]
            Z += _logsumexp64(pS + tcol, axis=0)
    return np.float32(Z), scores
